# revision 43
# baseline (speedup 1.0000x reference)
"""BKT-over-students kernel for Trainium2 (8 NeuronCores, data-parallel over B).

Math: the per-step BKT update
    correct_t = p(1-s) + (1-p)g
    k = p*a_y / (p*a_y + (1-p)*b_y)        a_1=1-s,b_1=g ; a_0=s,b_0=1-g
    p' = clip(k + (1-k)l, eps, 1-eps)
linearises in odds space v = p/(1-p):
    v' = A_t * v + B     with A_t = (a_y/b_y)/(1-l),  B = l/(1-l)
which maps 1:1 onto the DVE tensor_tensor_scan(op0=mult, op1=add)
instruction (one scan per 128 students covers all T steps).
The reference's lower clip never binds (v' >= B >= eps/(1-eps)); the upper
clip is enforced on the output side via p = 1 - 1/(1+v) which saturates to
1.0 (instead of NaN) when v overflows to inf, matching the reference's
clamped trajectory to ~1e-6 abs (saturation is absorbing here: A_t > 1).

Layout: device student d = 8*p + c (partition p, chunk c) so the y DMA and
both output DMAs see 8 consecutive DRAM rows per partition (32KB/16KB
contiguous runs -> 128 descriptors per DMA instead of 1024).  y ships as
int8 (values are 0/1, lossless).  The embedding gather happens host-side
(2 MB of the 44 MB total IO); the MLP runs on device; its last layer uses
lhsT = h2T so params land students-on-partitions with no PE transposes.
PE instructions carry a single semaphore wait, so every PE input is
funnelled through DVE.
"""

import numpy as np

import concourse.bacc as bacc
import concourse.tile as tile
from concourse import mybir
from concourse.bass_utils import run_bass_kernel_spmd

NCORES = 8
B, T = 8192, 1024
BC = B // NCORES          # students per core
P = 128
NCHUNK = BC // P          # 128-student chunks per core
H = 64                    # hidden dim
NOUT = 4                  # l, g, s, prior
EPS = 1e-6
F32 = mybir.dt.float32
I8 = mybir.dt.int8
ALU = mybir.AluOpType
ACTF = mybir.ActivationFunctionType
NWB = 2 * H + NOUT + 2    # packed weights: W0 | W1 | Wout | b0 | b1


def _build_bass():
    nc = bacc.Bacc("TRN2", target_bir_lowering=False, debug=False, num_devices=NCORES)

    y = nc.declare_dram_parameter("y", [BC, T], I8, isOutput=False)
    hT_in = nc.declare_dram_parameter("hT", [H, BC], F32, isOutput=False)
    wb = nc.declare_dram_parameter("wb", [H, NWB], F32, isOutput=False)
    bout = nc.declare_dram_parameter("bout", [1, NOUT], F32, isOutput=False)
    corrects = nc.declare_dram_parameter("corrects", [BC, T], F32, isOutput=True)
    latents = nc.declare_dram_parameter("latents", [BC, T], F32, isOutput=True)
    # DRAM row r = student d = 8*p + c  (partition p, chunk c)
    y3 = y.rearrange("(p c) t -> p c t", p=P, c=NCHUNK)
    lat3 = latents.rearrange("(p c) t -> p c t", p=P, c=NCHUNK)
    cor3 = corrects.rearrange("(p c) t -> p c t", p=P, c=NCHUNK)

    with tile.TileContext(nc) as tc:
        with (
            tc.tile_pool(name="singles", bufs=1) as singles,
            tc.tile_pool(name="psum", bufs=1, space="PSUM") as psum,
            tc.tile_pool(name="work", bufs=7) as work,
        ):
            # ---- inputs ----
            wbd = singles.tile([H, NWB], F32)
            nc.sync.dma_start(out=wbd[:], in_=wb[:])
            hTd = singles.tile([H, BC], F32)
            nc.sync.dma_start(out=hTd[:], in_=hT_in[:])
            boutb = singles.tile([P, NOUT], F32)
            nc.scalar.dma_start(out=boutb[:], in_=bout[:].to_broadcast([P, NOUT]))
            yt = singles.tile([P, NCHUNK * T], I8)
            nc.sync.dma_start(
                out=yt[:].rearrange("p (c t) -> p c t", c=NCHUNK),
                in_=y3,
            )

            # wb and hT arrive on the same DMA queue, so matmuls reading them
            # still carry a single wait; no DVE staging needed (bacc splits
            # any residual multi-waits into event semaphores).
            hT = hTd
            w0s = wbd[:, 0:H]
            w1s = wbd[:, H : 2 * H]
            wouts = wbd[:, 2 * H : 2 * H + NOUT]
            b0s = wbd[:, 2 * H + NOUT : 2 * H + NOUT + 1]
            b1s = wbd[:, 2 * H + NOUT + 1 : 2 * H + NOUT + 2]

            # PE p-state warmup: junk matmuls so the real MLP runs at speed
            wscr = singles.tile([H, 512], F32)
            nc.gpsimd.memset(wscr[:], 1.0)
            zw = psum.tile([H, 512], F32, tag="zw")
            for _ in range(2):
                nc.tensor.matmul(out=zw[:], lhsT=wscr[:, 0:H], rhs=wscr[:], start=True, stop=True)

            # ---- MLP layers 1-2 (students on free dim, DVE evacuation) ----
            h1T = singles.tile([H, BC], F32)
            h2T = singles.tile([H, BC], F32)
            NMM = 512
            for c in range(BC // NMM):
                sl = slice(c * NMM, (c + 1) * NMM)
                z1 = psum.tile([H, NMM], F32, tag="z1")
                nc.tensor.matmul(out=z1[:], lhsT=w0s, rhs=hT[:, sl], start=True, stop=True)
                nc.vector.tensor_scalar(
                    out=h1T[:, sl], in0=z1[:], scalar1=b0s, scalar2=0.0,
                    op0=ALU.add, op1=ALU.max,
                )
                z2 = psum.tile([H, NMM], F32, tag="z2")
                nc.tensor.matmul(out=z2[:], lhsT=w1s, rhs=h1T[:, sl], start=True, stop=True)
                nc.vector.tensor_scalar(
                    out=h2T[:, sl], in0=z2[:], scalar1=b1s, scalar2=0.0,
                    op0=ALU.add, op1=ALU.max,
                )

            # ---- per 2-chunk group: L3, derived constants, scans, stores ----
            GC = 8                       # chunks per group
            def pcols(t, k, grp):
                """(P, GC) view of param k, chunks grp*GC..grp*GC+GC-1."""
                return (
                    t[:, grp * GC * NOUT : (grp + 1) * GC * NOUT]
                    .rearrange("p (c k) -> p k c", k=NOUT)[:, k : k + 1, :]
                    .rearrange("p one c -> p (one c)")
                )

            ptall = singles.tile([P, NCHUNK * NOUT], F32)
            om = singles.tile([P, NCHUNK * NOUT], F32)
            rp = singles.tile([P, NCHUNK * NOUT], F32)
            rom = singles.tile([P, NCHUNK * NOUT], F32)
            da = singles.tile([P, NCHUNK], F32)   # A1 - A0
            a0t = singles.tile([P, NCHUNK], F32)  # A0
            bbt = singles.tile([P, NCHUNK], F32)  # B
            v0t = singles.tile([P, NCHUNK], F32)  # prior odds
            dsg = singles.tile([P, NCHUNK], F32)  # (1-s) - g
            oms = singles.tile([P, NCHUNK], F32)  # 1-s

            for grp in range(NCHUNK // GC):
                chunks = range(grp * GC, (grp + 1) * GC)
                hsl4 = slice(grp * GC * NOUT, (grp + 1) * GC * NOUT)
                hsl = slice(grp * GC, (grp + 1) * GC)

                # -- L3: params for this group's chunks, students on partitions --
                for c in chunks:
                    z3 = psum.tile([P, NOUT], F32, tag="z3")
                    nc.tensor.matmul(
                        out=z3[:], lhsT=h2T[:, c * P : (c + 1) * P], rhs=wouts,
                        start=True, stop=True,
                    )
                    zb = work.tile([P, NOUT], F32, tag="zb")
                    nc.vector.tensor_tensor(out=zb[:], in0=z3[:], in1=boutb[:], op=ALU.add)
                    nc.scalar.activation(
                        out=ptall[:, c * NOUT : (c + 1) * NOUT], in_=zb[:],
                        func=ACTF.Sigmoid,
                    )
                # clip params to [EPS, 1-EPS]
                nc.vector.tensor_scalar(
                    out=ptall[:, hsl4], in0=ptall[:, hsl4], scalar1=EPS,
                    scalar2=1.0 - EPS, op0=ALU.max, op1=ALU.min,
                )

                # -- derived constants (batched over the group's chunks) --
                nc.vector.tensor_scalar(
                    out=om[:, hsl4], in0=ptall[:, hsl4], scalar1=-1.0, scalar2=1.0,
                    op0=ALU.mult, op1=ALU.add,
                )
                nc.vector.reciprocal(out=rp[:, hsl4], in_=ptall[:, hsl4])
                nc.vector.reciprocal(out=rom[:, hsl4], in_=om[:, hsl4])
                # A1 = (1-s)/(g*(1-l));  A0 = s/((1-g)*(1-l))
                nc.vector.tensor_tensor(out=da[:, hsl], in0=pcols(om, 2, grp), in1=pcols(rp, 1, grp), op=ALU.mult)
                nc.vector.tensor_tensor(out=da[:, hsl], in0=da[:, hsl], in1=pcols(rom, 0, grp), op=ALU.mult)
                nc.vector.tensor_tensor(out=a0t[:, hsl], in0=pcols(ptall, 2, grp), in1=pcols(rom, 1, grp), op=ALU.mult)
                nc.vector.tensor_tensor(out=a0t[:, hsl], in0=a0t[:, hsl], in1=pcols(rom, 0, grp), op=ALU.mult)
                nc.vector.tensor_tensor(out=da[:, hsl], in0=da[:, hsl], in1=a0t[:, hsl], op=ALU.subtract)
                nc.vector.tensor_tensor(out=bbt[:, hsl], in0=pcols(ptall, 0, grp), in1=pcols(rom, 0, grp), op=ALU.mult)
                nc.vector.tensor_tensor(out=v0t[:, hsl], in0=pcols(ptall, 3, grp), in1=pcols(rom, 3, grp), op=ALU.mult)
                # dsg holds g-(1-s) = -((1-s)-g): corrects = (1-s) + dsg*rr
                nc.vector.tensor_tensor(out=dsg[:, hsl], in0=pcols(ptall, 1, grp), in1=pcols(om, 2, grp), op=ALU.subtract)
                nc.vector.tensor_copy(out=oms[:, hsl], in_=pcols(om, 2, grp))

                # -- scans + outputs for this group (per-chunk stores) --
                for j, c in enumerate(chunks):
                    pph = work.tile([P, T], F32, tag="pp2")
                    crh = work.tile([P, T], F32, tag="cr2")
                    ysl = yt[:, c * T : (c + 1) * T]
                    # A_t = y*dA + A0  (>0, so Relu is a no-op; int8 in, f32 out)
                    at = work.tile([P, T], F32, tag="at")
                    nc.scalar.activation(
                        out=at[:], in_=ysl, func=ACTF.Relu,
                        scale=da[:, c : c + 1], bias=a0t[:, c : c + 1],
                    )
                    # L[:, t] = odds before step t;  L[:, 0] = prior odds
                    ll = work.tile([P, T], F32, tag="ll")
                    nc.gpsimd.tensor_copy(out=ll[:, 0:1], in_=v0t[:, c : c + 1])
                    nc.vector.tensor_tensor_scan(
                        out=ll[:, 1:T], data0=at[:, 0 : T - 1],
                        data1=bbt[:, c : c + 1].to_broadcast([P, T - 1]),
                        initial=v0t[:, c : c + 1], op0=ALU.mult, op1=ALU.add,
                    )
                    # dd = min(v, 1e30) + 1: the min guards recip_approx_fast,
                    # whose behaviour at inf is undefined
                    dd = work.tile([P, T], F32, tag="dd")
                    nc.gpsimd.tensor_scalar(
                        out=dd[:], in0=ll[:], scalar1=1e30, scalar2=1.0,
                        op0=ALU.min, op1=ALU.add,
                    )
                    rr = work.tile([P, T], F32, tag="rr")
                    nc.vector.reciprocal_approx_fast(out=rr[:], in_=dd[:])
                    # latents p = 1 - 1/(1+v)   (v>=1e30 -> 1.0, no NaN)
                    psl = pph[:, 0:T]
                    if c % 2 == 0:
                        nc.scalar.activation(
                            out=psl, in_=rr[:], func=ACTF.Copy, scale=-1.0, bias=1.0,
                        )
                    else:
                        nc.gpsimd.tensor_scalar(
                            out=psl, in0=rr[:], scalar1=-1.0, scalar2=1.0,
                            op0=ALU.mult, op1=ALU.add,
                        )
                    # corrects = (1-s) + dsg/(1+v)  with dsg = g-(1-s), from rr
                    nc.scalar.activation(
                        out=crh[:, 0:T], in_=rr[:],
                        func=ACTF.Relu,
                        scale=dsg[:, c : c + 1], bias=oms[:, c : c + 1],
                    )
                    sl1 = slice(c, c + 1)
                    eng_l = nc.sync if c % 2 == 0 else nc.scalar
                    eng_c = nc.scalar if c % 2 == 0 else nc.sync
                    eng_l.dma_start(
                        out=lat3[:, sl1, :],
                        in_=pph[:].rearrange("p (c t) -> p c t", c=1),
                    )
                    eng_c.dma_start(
                        out=cor3[:, sl1, :],
                        in_=crh[:].rearrange("p (c t) -> p c t", c=1),
                    )
    nc.compile()
    return nc


_NC_CACHE = None


def _get_nc():
    global _NC_CACHE
    if _NC_CACHE is None:
        _NC_CACHE = _build_bass()
    return _NC_CACHE


def kernel(X, y, embed, W0, b0, W1, b1, Wout, bout):
    X = np.asarray(X).astype(np.int64)
    y8 = np.asarray(y, dtype=np.int8)
    embed = np.asarray(embed, dtype=np.float32)
    W0 = np.asarray(W0, dtype=np.float32)
    W1 = np.asarray(W1, dtype=np.float32)
    Wout = np.asarray(Wout, dtype=np.float32)
    b0 = np.asarray(b0, dtype=np.float32).reshape(H)
    b1 = np.asarray(b1, dtype=np.float32).reshape(H)
    bout_v = np.asarray(bout, dtype=np.float32).reshape(1, NOUT)

    h = embed[X]                                   # (B, H) host-side gather
    wb_pack = np.ascontiguousarray(
        np.concatenate([W0, W1, Wout, b0[:, None], b1[:, None]], axis=1)
        .astype(np.float32)
    )

    # Device chunk c holds students {8p + c}; hT column c*128+p must be
    # student 8p+c, so permute the gather result accordingly per core.
    perm = np.concatenate([np.arange(P) * NCHUNK + c for c in range(NCHUNK)])
    nc = _get_nc()
    in_maps = []
    for c in range(NCORES):
        rows = slice(c * BC, (c + 1) * BC)
        in_maps.append({
            "y": np.ascontiguousarray(y8[rows]),
            "hT": np.ascontiguousarray(h[rows][perm].T),
            "wb": wb_pack,
            "bout": bout_v,
        })
    res = run_bass_kernel_spmd(nc, in_maps, list(range(NCORES)))
    corrects = np.concatenate([res.results[c]["corrects"] for c in range(NCORES)], axis=0)
    latents = np.concatenate([res.results[c]["latents"] for c in range(NCORES)], axis=0)
    return corrects, latents


# revision 46
# speedup vs baseline: 1.0182x; 1.0182x over previous
"""BKT-over-students kernel for Trainium2 (8 NeuronCores, data-parallel over B).

Math: the per-step BKT update
    correct_t = p(1-s) + (1-p)g
    k = p*a_y / (p*a_y + (1-p)*b_y)        a_1=1-s,b_1=g ; a_0=s,b_0=1-g
    p' = clip(k + (1-k)l, eps, 1-eps)
linearises in odds space v = p/(1-p):
    v' = A_t * v + B     with A_t = (a_y/b_y)/(1-l),  B = l/(1-l)
which maps 1:1 onto the DVE tensor_tensor_scan(op0=mult, op1=add)
instruction (one scan per 128 students covers all T steps).
The reference's lower clip never binds (v' >= B >= eps/(1-eps)); the upper
clip is enforced on the output side via p = 1 - 1/(1+v) which saturates to
1.0 (instead of NaN) when v overflows to inf, matching the reference's
clamped trajectory to ~1e-6 abs (saturation is absorbing here: A_t > 1).

Layout: device student d = 8*p + c (partition p, chunk c) so the y DMA and
both output DMAs see 8 consecutive DRAM rows per partition (32KB/16KB
contiguous runs -> 128 descriptors per DMA instead of 1024).  y ships as
int8 (values are 0/1, lossless).  The embedding gather happens host-side
(2 MB of the 44 MB total IO); the MLP runs on device; its last layer uses
lhsT = h2T so params land students-on-partitions with no PE transposes.
PE instructions carry a single semaphore wait, so every PE input is
funnelled through DVE.
"""

import numpy as np

import concourse.bacc as bacc
import concourse.tile as tile
from concourse import mybir
from concourse.bass_utils import run_bass_kernel_spmd

NCORES = 8
B, T = 8192, 1024
BC = B // NCORES          # students per core
P = 128
NCHUNK = BC // P          # 128-student chunks per core
H = 64                    # hidden dim
NOUT = 4                  # l, g, s, prior
EPS = 1e-6
F32 = mybir.dt.float32
I8 = mybir.dt.int8
ALU = mybir.AluOpType
ACTF = mybir.ActivationFunctionType
NWB = 2 * H + NOUT + 2    # packed weights: W0 | W1 | Wout | b0 | b1


def _build_bass():
    nc = bacc.Bacc("TRN2", target_bir_lowering=False, debug=False, num_devices=NCORES)

    y = nc.declare_dram_parameter("y", [BC, T], I8, isOutput=False)
    hT_in = nc.declare_dram_parameter("hT", [H, BC], F32, isOutput=False)
    wb = nc.declare_dram_parameter("wb", [H, NWB], F32, isOutput=False)
    bout = nc.declare_dram_parameter("bout", [1, NOUT], F32, isOutput=False)
    corrects = nc.declare_dram_parameter("corrects", [BC, T], F32, isOutput=True)
    latents = nc.declare_dram_parameter("latents", [BC, T], F32, isOutput=True)
    # DRAM row r = student d = 8*p + c  (partition p, chunk c)
    y3 = y.rearrange("(p c) t -> p c t", p=P, c=NCHUNK)
    lat3 = latents.rearrange("(p c) t -> p c t", p=P, c=NCHUNK)
    cor3 = corrects.rearrange("(p c) t -> p c t", p=P, c=NCHUNK)

    with tile.TileContext(nc) as tc:
        with (
            tc.tile_pool(name="singles", bufs=1) as singles,
            tc.tile_pool(name="psum", bufs=1, space="PSUM") as psum,
            tc.tile_pool(name="work", bufs=7) as work,
        ):
            # ---- inputs ----
            wbd = singles.tile([H, NWB], F32)
            nc.sync.dma_start(out=wbd[:], in_=wb[:])
            hTd = singles.tile([H, BC], F32)
            nc.sync.dma_start(out=hTd[:, 0:512], in_=hT_in[:, 0:512])
            nc.sync.dma_start(out=hTd[:, 512:BC], in_=hT_in[:, 512:BC])
            boutb = singles.tile([P, NOUT], F32)
            nc.scalar.dma_start(out=boutb[:], in_=bout[:].to_broadcast([P, NOUT]))
            yt = singles.tile([P, NCHUNK * T], I8)
            nc.sync.dma_start(
                out=yt[:].rearrange("p (c t) -> p c t", c=NCHUNK),
                in_=y3,
            )

            # wb and hT arrive on the same DMA queue, so matmuls reading them
            # still carry a single wait; no DVE staging needed (bacc splits
            # any residual multi-waits into event semaphores).
            hT = hTd
            w0s = wbd[:, 0:H]
            w1s = wbd[:, H : 2 * H]
            wouts = wbd[:, 2 * H : 2 * H + NOUT]
            b0s = wbd[:, 2 * H + NOUT : 2 * H + NOUT + 1]
            b1s = wbd[:, 2 * H + NOUT + 1 : 2 * H + NOUT + 2]

            # PE p-state warmup: junk matmuls so the real MLP runs at speed
            wscr = singles.tile([H, 512], F32)
            nc.gpsimd.memset(wscr[:], 1.0)
            zw = psum.tile([H, 512], F32, tag="zw")
            for _ in range(2):
                nc.tensor.matmul(out=zw[:], lhsT=wscr[:, 0:H], rhs=wscr[:], start=True, stop=True)

            # ---- MLP layers 1-2 (students on free dim, DVE evacuation) ----
            h1T = singles.tile([H, BC], F32)
            h2T = singles.tile([H, BC], F32)
            NMM = 512
            for c in range(BC // NMM):
                sl = slice(c * NMM, (c + 1) * NMM)
                z1 = psum.tile([H, NMM], F32, tag="z1")
                nc.tensor.matmul(out=z1[:], lhsT=w0s, rhs=hT[:, sl], start=True, stop=True)
                nc.scalar.activation(out=h1T[:, sl], in_=z1[:], func=ACTF.Relu, bias=b0s)
                z2 = psum.tile([H, NMM], F32, tag="z2")
                nc.tensor.matmul(out=z2[:], lhsT=w1s, rhs=h1T[:, sl], start=True, stop=True)
                nc.scalar.activation(out=h2T[:, sl], in_=z2[:], func=ACTF.Relu, bias=b1s)

            # ---- per 2-chunk group: L3, derived constants, scans, stores ----
            GC = 8                       # chunks per group
            def pcols(t, k, grp):
                """(P, GC) view of param k, chunks grp*GC..grp*GC+GC-1."""
                return (
                    t[:, grp * GC * NOUT : (grp + 1) * GC * NOUT]
                    .rearrange("p (c k) -> p k c", k=NOUT)[:, k : k + 1, :]
                    .rearrange("p one c -> p (one c)")
                )

            ptall = singles.tile([P, NCHUNK * NOUT], F32)
            om = singles.tile([P, NCHUNK * NOUT], F32)
            rp = singles.tile([P, NCHUNK * NOUT], F32)
            rom = singles.tile([P, NCHUNK * NOUT], F32)
            da = singles.tile([P, NCHUNK], F32)   # A1 - A0
            a0t = singles.tile([P, NCHUNK], F32)  # A0
            bbt = singles.tile([P, NCHUNK], F32)  # B
            v0t = singles.tile([P, NCHUNK], F32)  # prior odds
            dsg = singles.tile([P, NCHUNK], F32)  # (1-s) - g
            oms = singles.tile([P, NCHUNK], F32)  # 1-s

            for grp in range(NCHUNK // GC):
                chunks = range(grp * GC, (grp + 1) * GC)
                hsl4 = slice(grp * GC * NOUT, (grp + 1) * GC * NOUT)
                hsl = slice(grp * GC, (grp + 1) * GC)

                # -- L3: params for this group's chunks, students on partitions --
                for c in chunks:
                    z3 = psum.tile([P, NOUT], F32, tag="z3")
                    nc.tensor.matmul(
                        out=z3[:], lhsT=h2T[:, c * P : (c + 1) * P], rhs=wouts,
                        start=True, stop=True,
                    )
                    zb = work.tile([P, NOUT], F32, tag="zb")
                    nc.vector.tensor_tensor(out=zb[:], in0=z3[:], in1=boutb[:], op=ALU.add)
                    nc.scalar.activation(
                        out=ptall[:, c * NOUT : (c + 1) * NOUT], in_=zb[:],
                        func=ACTF.Sigmoid,
                    )
                # clip params to [EPS, 1-EPS]
                nc.vector.tensor_scalar(
                    out=ptall[:, hsl4], in0=ptall[:, hsl4], scalar1=EPS,
                    scalar2=1.0 - EPS, op0=ALU.max, op1=ALU.min,
                )

                # -- derived constants (batched over the group's chunks) --
                nc.vector.tensor_scalar(
                    out=om[:, hsl4], in0=ptall[:, hsl4], scalar1=-1.0, scalar2=1.0,
                    op0=ALU.mult, op1=ALU.add,
                )
                nc.vector.reciprocal(out=rp[:, hsl4], in_=ptall[:, hsl4])
                nc.vector.reciprocal(out=rom[:, hsl4], in_=om[:, hsl4])
                # A1 = (1-s)/(g*(1-l));  A0 = s/((1-g)*(1-l))
                nc.vector.tensor_tensor(out=da[:, hsl], in0=pcols(om, 2, grp), in1=pcols(rp, 1, grp), op=ALU.mult)
                nc.vector.tensor_tensor(out=da[:, hsl], in0=da[:, hsl], in1=pcols(rom, 0, grp), op=ALU.mult)
                nc.vector.tensor_tensor(out=a0t[:, hsl], in0=pcols(ptall, 2, grp), in1=pcols(rom, 1, grp), op=ALU.mult)
                nc.vector.tensor_tensor(out=a0t[:, hsl], in0=a0t[:, hsl], in1=pcols(rom, 0, grp), op=ALU.mult)
                nc.vector.tensor_tensor(out=da[:, hsl], in0=da[:, hsl], in1=a0t[:, hsl], op=ALU.subtract)
                nc.vector.tensor_tensor(out=bbt[:, hsl], in0=pcols(ptall, 0, grp), in1=pcols(rom, 0, grp), op=ALU.mult)
                nc.vector.tensor_tensor(out=v0t[:, hsl], in0=pcols(ptall, 3, grp), in1=pcols(rom, 3, grp), op=ALU.mult)
                # dsg holds g-(1-s) = -((1-s)-g): corrects = (1-s) + dsg*rr
                nc.vector.tensor_tensor(out=dsg[:, hsl], in0=pcols(ptall, 1, grp), in1=pcols(om, 2, grp), op=ALU.subtract)
                nc.vector.tensor_copy(out=oms[:, hsl], in_=pcols(om, 2, grp))

                # -- scans + outputs for this group (per-chunk stores) --
                for j, c in enumerate(chunks):
                    pph = work.tile([P, T], F32, tag="pp2")
                    crh = work.tile([P, T], F32, tag="cr2")
                    ysl = yt[:, c * T : (c + 1) * T]
                    # A_t = y*dA + A0  (>0, so Relu is a no-op; int8 in, f32 out)
                    at = work.tile([P, T], F32, tag="at")
                    nc.scalar.activation(
                        out=at[:], in_=ysl, func=ACTF.Relu,
                        scale=da[:, c : c + 1], bias=a0t[:, c : c + 1],
                    )
                    # L[:, t] = odds before step t;  L[:, 0] = prior odds
                    ll = work.tile([P, T], F32, tag="ll")
                    nc.gpsimd.tensor_copy(out=ll[:, 0:1], in_=v0t[:, c : c + 1])
                    nc.vector.tensor_tensor_scan(
                        out=ll[:, 1:T], data0=at[:, 0 : T - 1],
                        data1=bbt[:, c : c + 1].to_broadcast([P, T - 1]),
                        initial=v0t[:, c : c + 1], op0=ALU.mult, op1=ALU.add,
                    )
                    # dd = min(v, 1e30) + 1: the min guards recip_approx_fast,
                    # whose behaviour at inf is undefined.  The final chunk
                    # keeps its whole chain on DVE (no cross-engine hops on
                    # the critical tail).
                    dd = work.tile([P, T], F32, tag="dd")
                    dd_eng = nc.vector if c == NCHUNK - 1 else nc.gpsimd
                    dd_eng.tensor_scalar(
                        out=dd[:], in0=ll[:], scalar1=1e30, scalar2=1.0,
                        op0=ALU.min, op1=ALU.add,
                    )
                    rr = work.tile([P, T], F32, tag="rr")
                    nc.vector.reciprocal_approx_fast(out=rr[:], in_=dd[:])
                    # latents p = 1 - 1/(1+v)   (v>=1e30 -> 1.0, no NaN)
                    psl = pph[:, 0:T]
                    if c % 2 == 0:
                        nc.scalar.activation(
                            out=psl, in_=rr[:], func=ACTF.Copy, scale=-1.0, bias=1.0,
                        )
                    else:
                        nc.gpsimd.tensor_scalar(
                            out=psl, in0=rr[:], scalar1=-1.0, scalar2=1.0,
                            op0=ALU.mult, op1=ALU.add,
                        )
                    # corrects = (1-s) + dsg/(1+v)  with dsg = g-(1-s), from rr
                    nc.scalar.activation(
                        out=crh[:, 0:T], in_=rr[:],
                        func=ACTF.Relu,
                        scale=dsg[:, c : c + 1], bias=oms[:, c : c + 1],
                    )
                    sl1 = slice(c, c + 1)
                    eng_l = nc.sync if c % 2 == 0 else nc.scalar
                    eng_c = nc.scalar if c % 2 == 0 else nc.sync
                    eng_l.dma_start(
                        out=lat3[:, sl1, :],
                        in_=pph[:].rearrange("p (c t) -> p c t", c=1),
                    )
                    eng_c.dma_start(
                        out=cor3[:, sl1, :],
                        in_=crh[:].rearrange("p (c t) -> p c t", c=1),
                    )
    nc.compile()
    return nc


_NC_CACHE = None


def _get_nc():
    global _NC_CACHE
    if _NC_CACHE is None:
        _NC_CACHE = _build_bass()
    return _NC_CACHE


def kernel(X, y, embed, W0, b0, W1, b1, Wout, bout):
    X = np.asarray(X).astype(np.int64)
    y8 = np.asarray(y, dtype=np.int8)
    embed = np.asarray(embed, dtype=np.float32)
    W0 = np.asarray(W0, dtype=np.float32)
    W1 = np.asarray(W1, dtype=np.float32)
    Wout = np.asarray(Wout, dtype=np.float32)
    b0 = np.asarray(b0, dtype=np.float32).reshape(H)
    b1 = np.asarray(b1, dtype=np.float32).reshape(H)
    bout_v = np.asarray(bout, dtype=np.float32).reshape(1, NOUT)

    h = embed[X]                                   # (B, H) host-side gather
    wb_pack = np.ascontiguousarray(
        np.concatenate([W0, W1, Wout, b0[:, None], b1[:, None]], axis=1)
        .astype(np.float32)
    )

    # Device chunk c holds students {8p + c}; hT column c*128+p must be
    # student 8p+c, so permute the gather result accordingly per core.
    perm = np.concatenate([np.arange(P) * NCHUNK + c for c in range(NCHUNK)])
    nc = _get_nc()
    in_maps = []
    for c in range(NCORES):
        rows = slice(c * BC, (c + 1) * BC)
        in_maps.append({
            "y": np.ascontiguousarray(y8[rows]),
            "hT": np.ascontiguousarray(h[rows][perm].T),
            "wb": wb_pack,
            "bout": bout_v,
        })
    res = run_bass_kernel_spmd(nc, in_maps, list(range(NCORES)))
    corrects = np.concatenate([res.results[c]["corrects"] for c in range(NCORES)], axis=0)
    latents = np.concatenate([res.results[c]["latents"] for c in range(NCORES)], axis=0)
    return corrects, latents
